# revision 38
# baseline (speedup 1.0000x reference)
"""Sparse L1-distance attention (nn_L1AttnSparse) on 8 Trainium2 NeuronCores.

Sharding: dst tokens split across the 8 cores (256 dst each, x2 batches).
Per (batch, 128-dst chunk) iteration the 4096 edges (128 dst x 32 slots)
are processed as:

  - k rows gathered fp16 TRANSPOSED ([w-lane partitions, edge free], 8
    gathers of 512 edges) so the per-(slot,head) |q-k| sum over w runs on
    the PE array: stationary = |kT - qT| slot-plane [128 w-lanes, 128 dst],
    moving = an 8-col head mask; PSUM accumulates the 4 w-lane groups and
    scores land directly as [128 dst, (slot, head)]. The subtract is a DVE
    2x fp16 TensorTensor; the abs runs on the ACT engine.
  - softmax skips the max-subtraction (L = sum|q-k|/8 <= ~25 so exp(-L)
    stays inside fp32 range): per-granule exp on ACT + sum + reciprocal.
  - v rows gathered fp16 untransposed in w-major row layout (4 gathers of
    1024 edges) so the weight broadcast keeps every operand innermost-
    packed and the weighting runs in the DVE 2x fp16 mode; the slot sum is
    32 identity-stationary matmuls accumulating into PSUM on the PE, and
    the 1/den normalization is applied once on the PSUM readout.

k and q are pre-scaled by 1/8 (= 1/sqrt(64)) on the host. All gathers stay
at <= 1024 indices (<= 512 transposed): larger SWDGE descriptor batches
crash the runtime.
"""

import sys

sys.path.insert(0, "/opt/trn_rl_repo")

import numpy as np

import concourse.bass as bass
import concourse.tile as tile
from concourse import bacc, mybir
from concourse.bass_utils import run_bass_kernel_spmd

BS = 2
N_TOK = 2048
NH = 8
W = 64
S = 32  # dst_mxlen
HW = NH * W  # 512 values per token row
N_CORES = 8
DT = N_TOK // N_CORES  # dst tokens per core = 256
CHUNKS = DT // 128  # dst chunks of 128 per core = 2
CQ = HW // 128  # 4 w-lane groups (transposed gather rows per partition)
KG = 8  # k gathers per chunk (512 edges each = 4 slots)
KS = S // KG  # 4 slots per k gather
KNI = 128 * KS  # 512
VG = 4  # v gathers per chunk (1024 edges each = 8 slots)
VS = S // VG  # 8 slots per v gather
VNI = 128 * VS  # 1024


def _wrap_idx(flat):
    """int16 index list -> [128, n/16] tile layout: idx i at [i%16, i//16],
    replicated down the 8 groups of 16 partitions."""
    n = flat.shape[0]
    w16 = np.zeros((16, n // 16), dtype=np.int16)
    w16[np.arange(n) % 16, np.arange(n) // 16] = flat
    return np.tile(w16, (8, 1))


def build_kernel():
    nc = bacc.Bacc(
        "TRN2", target_bir_lowering=False, debug=False, num_devices=N_CORES,
        dynamic_dma_scratch_size=16384 * 4,
    )
    f16 = mybir.dt.float16
    f32 = mybir.dt.float32
    i16 = mybir.dt.int16
    A = mybir.AluOpType

    kt = nc.dram_tensor("kt", [BS * N_TOK, HW], f16, kind="ExternalInput").ap()
    vt = nc.dram_tensor("vt", [BS * N_TOK, HW], f16, kind="ExternalInput").ap()
    # cst packs [qT (b c n) | msk | idn] along the free dim per partition
    NIT = BS * CHUNKS
    CSTW = NIT * HW + CQ * NH + 128
    cst = nc.dram_tensor("cst", [128, CSTW], f16, kind="ExternalInput").ap()
    idx = nc.dram_tensor(
        "idx", [BS, CHUNKS, 128, S * 128 // 16], i16, kind="ExternalInput"
    ).ap()
    oc = nc.dram_tensor("oc", [BS, CHUNKS, 128, HW], f16, kind="ExternalOutput").ap()

    with tile.TileContext(nc) as tc:
        with (
            tc.tile_pool(name="kp", bufs=8) as kp,       # 0.5MB k gather tiles
            tc.tile_pool(name="vp", bufs=5) as vp,       # 1MB v gather tiles
            tc.tile_pool(name="sp", bufs=10) as sp,      # small tiles
            tc.tile_pool(name="ip", bufs=3) as ip,       # idx tiles
            tc.psum_pool(name="pp", bufs=2) as pp,       # scores
        ):
            it_all = ip.tile([128, NIT, S * 128 // 16], i16, tag="idx", bufs=1)
            nc.sync.dma_start(
                out=it_all[:], in_=idx.rearrange("b c p n -> p (b c) n")
            )
            cst_t = sp.tile([128, CSTW], f16, tag="cst", bufs=1)
            nc.sync.dma_start(out=cst_t[:], in_=cst)
            q_all = cst_t[:, : NIT * HW].rearrange("p (i n) -> p i n", n=HW)
            msk_t = cst_t[:, NIT * HW : NIT * HW + CQ * NH]
            id_t = cst_t[:, NIT * HW + CQ * NH :]

            for b in range(BS):
                for c in range(CHUNKS):
                    bc = b * CHUNKS + c
                    it = it_all[:, bc]
                    itv = it_all[:, bc].rearrange("p (g n) -> p g n", n=KNI // 16)
                    qv = q_all[:, bc].rearrange("p (cq d) -> p cq d", d=128)[
                        :, :, None, :
                    ].to_broadcast([128, CQ, KS, 128])

                    psum = pp.tile([128, S * NH], f32, tag="L")
                    E = sp.tile([128, S * NH], f16, tag="E")

                    def k_granule(g):
                        kg = kp.tile([128, CQ, KNI], f16, tag="kg")
                        nc.gpsimd.dma_gather(
                            kg[:], kt, itv[:, g], KNI, KNI, HW,
                            transpose=True, queue_num=0,
                        )
                        kv4 = kg[:].rearrange("p cq (s d) -> p cq s d", d=128)
                        # D = k/8 - q/8 (in place on the gather tile)
                        nc.vector.tensor_tensor(
                            out=kv4, in0=kv4, in1=qv, op=A.subtract,
                        )
                        # |D| in place on the ACT engine
                        nc.scalar.activation(
                            out=kg[:], in_=kg[:],
                            func=mybir.ActivationFunctionType.Abs,
                        )
                        # PE: L[d, (s h)] += sum_w |D| via head-mask matmuls
                        for s in range(KS):
                            sl = (g * KS + s) * NH
                            for cq in range(CQ):
                                nc.tensor.matmul(
                                    psum[:, sl : sl + NH],
                                    kv4[:, cq, s, :],
                                    msk_t[:, cq * NH : (cq + 1) * NH],
                                    start=(cq == 0),
                                    stop=(cq == CQ - 1),
                                )
                        # early per-granule exp so the v side can start
                        # before the remaining score granules finish
                        gs = g * KS * NH
                        nc.scalar.activation(
                            out=E[:, gs : gs + KS * NH],
                            in_=psum[:, gs : gs + KS * NH],
                            func=mybir.ActivationFunctionType.Exp, scale=-1.0,
                        )

                    psum_o = pp.tile([128, HW], f32, tag="O")

                    def v_granule(s0, ns):
                        # gathers slots [s0, s0+ns), ns*128 indices
                        vg = vp.tile([128, ns, HW], f16, tag=f"vg{ns}")
                        nc.gpsimd.dma_gather(
                            vg[:], vt,
                            it_all[:, bc].rearrange(
                                "p (s n) -> p s n", n=128 // 16
                            )[:, s0 : s0 + ns].rearrange("p s n -> p (s n)"),
                            ns * 128, ns * 128, HW, queue_num=0,
                        )
                        ev = E[:, s0 * NH : (s0 + ns) * NH].rearrange(
                            "p (s h) -> p s h", h=NH
                        )[:, :, None, :].to_broadcast([128, ns, W, NH])
                        # weighted products in place
                        nc.vector.tensor_tensor(
                            out=vg[:].rearrange("p s (w h) -> p s w h", h=NH),
                            in0=vg[:].rearrange("p s (w h) -> p s w h", h=NH),
                            in1=ev,
                            op=A.mult,
                        )
                        # slot sum on PE: psum_o += I @ P_s
                        for s in range(ns):
                            nc.tensor.matmul(
                                psum_o[:],
                                id_t,
                                vg[:, s],
                                start=(s0 + s == 0),
                                stop=(s0 + s == S - 1),
                                skip_group_check=True,
                            )

                    # k gathers first (their compute chain is the long pole),
                    # then the v stream lands against ready score granules
                    for g in range(KG):
                        k_granule(g)

                    # normalizer (needs all score granules; consumed by the
                    # final scale only, so it hides under the v stream)
                    den = sp.tile([128, NH], f32, tag="den")
                    nc.vector.tensor_reduce(
                        out=den[:],
                        in_=E[:].rearrange("p (s h) -> p h s", h=NH),
                        axis=mybir.AxisListType.X,
                        op=A.add,
                    )
                    rden = sp.tile([128, NH], f32, tag="rden")
                    nc.vector.reciprocal(rden[:], den[:])

                    last = b == BS - 1 and c == CHUNKS - 1
                    for g in range(VG):
                        if last and g == VG - 1:
                            # split the final granule to shorten the tail
                            v_granule(g * VS, 4)
                            v_granule(g * VS + 4, 2)
                            v_granule(g * VS + 6, 2)
                        else:
                            v_granule(g * VS, VS)

                    # normalize: out = psum_o * (1/den), broadcast over w
                    ot = sp.tile([128, HW], f16, tag="oc")
                    nc.vector.tensor_tensor(
                        out=ot[:].rearrange("p (w h) -> p w h", h=NH),
                        in0=psum_o[:].rearrange("p (w h) -> p w h", h=NH),
                        in1=rden[:][:, None, :].to_broadcast([128, W, NH]),
                        op=A.mult,
                    )
                    nc.sync.dma_start(out=oc[b, c], in_=ot[:])
    nc.compile()
    return nc


_NC_CACHE = None


def kernel(v, q, k, coo, dst_mxlen):
    global _NC_CACHE
    assert int(dst_mxlen) == S
    v = np.asarray(v, dtype=np.float32)
    q = np.asarray(q, dtype=np.float32)
    k = np.asarray(k, dtype=np.float32)
    coo = np.asarray(coo)

    # src table: srct[t, s] = src index of edge (dst=t, slot=s)
    srct = np.zeros((N_TOK, S), dtype=np.int64)
    srct[coo[:, 0], coo[:, 2]] = coo[:, 1]

    scale = 1.0 / np.sqrt(W)
    kt = (k * scale).astype(np.float16).reshape(BS * N_TOK, HW)
    # v table in w-major row layout: row[(w, h)] = v[h, w]
    vt = np.ascontiguousarray(v.transpose(0, 1, 3, 2)).astype(np.float16)
    vt = vt.reshape(BS * N_TOK, HW)
    qs = (q * scale).astype(np.float16).reshape(BS, N_TOK, HW)

    # PE head masks: msk[p, cq*8 + h] = 1 iff h == 2*cq + (p >= 64)
    msk = np.zeros((128, CQ * NH), dtype=np.float16)
    for cq in range(CQ):
        msk[0:64, cq * NH + 2 * cq] = 1.0
        msk[64:128, cq * NH + 2 * cq + 1] = 1.0

    if _NC_CACHE is None:
        _NC_CACHE = build_kernel()
    nc = _NC_CACHE

    in_maps = []
    for core in range(N_CORES):
        lo = core * DT
        # qT[p, (cq, d)] = q_scaled[b, lo + c*128 + d, cq*128 + p]
        qT = np.empty((BS, CHUNKS, 128, HW), dtype=np.float16)
        for b in range(BS):
            for c in range(CHUNKS):
                blk = qs[b, lo + c * 128 : lo + (c + 1) * 128]  # [128 d, 512]
                qT[b, c] = (
                    blk.reshape(128, CQ, 128).transpose(2, 1, 0).reshape(128, HW)
                )
        # edge i = s*128 + d; wrapped per 512-idx granule, concatenated
        idx = np.zeros((BS, CHUNKS, 128, S * 128 // 16), dtype=np.int16)
        for b in range(BS):
            for c in range(CHUNKS):
                flat = (
                    b * N_TOK + srct[lo + c * 128 : lo + (c + 1) * 128, :].T
                ).reshape(-1).astype(np.int16)  # [(s), (d)] flattened
                for g in range(KG):
                    idx[b, c, :, g * (KNI // 16) : (g + 1) * (KNI // 16)] = (
                        _wrap_idx(flat[g * KNI : (g + 1) * KNI])
                    )
        cst = np.concatenate(
            [
                qT.transpose(2, 0, 1, 3).reshape(128, BS * CHUNKS * HW),
                msk,
                np.eye(128, dtype=np.float16),
            ],
            axis=1,
        )
        in_maps.append({"kt": kt, "vt": vt, "cst": cst, "idx": idx})

    res = run_bass_kernel_spmd(nc, in_maps, list(range(N_CORES)))
    out = np.empty((BS, N_TOK, NH, W), dtype=np.float32)
    for core in range(N_CORES):
        lo = core * DT
        o = res.results[core]["oc"].astype(np.float32)  # [BS, CHUNKS, 128, (w h)]
        o = o.reshape(BS, CHUNKS, 128, W, NH).transpose(0, 1, 2, 4, 3)
        out[:, lo : lo + DT] = o.reshape(BS, DT, NH, W)
    return out


# revision 41
# speedup vs baseline: 1.0049x; 1.0049x over previous
"""Sparse L1-distance attention (nn_L1AttnSparse) on 8 Trainium2 NeuronCores.

Sharding: dst tokens split across the 8 cores (256 dst each, x2 batches).
Per (batch, 128-dst chunk) iteration the 4096 edges (128 dst x 32 slots)
are processed as:

  - k rows gathered fp16 TRANSPOSED ([w-lane partitions, edge free], 8
    gathers of 512 edges) so the per-(slot,head) |q-k| sum over w runs on
    the PE array: stationary = |kT - qT| slot-plane [128 w-lanes, 128 dst],
    moving = an 8-col head mask; PSUM accumulates the 4 w-lane groups and
    scores land directly as [128 dst, (slot, head)]. The subtract is a DVE
    2x fp16 TensorTensor; the abs runs on the ACT engine.
  - softmax skips the max-subtraction (L = sum|q-k|/8 <= ~25 so exp(-L)
    stays inside fp32 range): per-granule exp on ACT + sum + reciprocal.
  - v rows gathered fp16 untransposed in w-major row layout (4 gathers of
    1024 edges) so the weight broadcast keeps every operand innermost-
    packed and the weighting runs in the DVE 2x fp16 mode; the slot sum is
    32 identity-stationary matmuls accumulating into PSUM on the PE, and
    the 1/den normalization is applied once on the PSUM readout.

k and q are pre-scaled by 1/8 (= 1/sqrt(64)) on the host. All gathers stay
at <= 1024 indices (<= 512 transposed): larger SWDGE descriptor batches
crash the runtime.
"""

import sys

sys.path.insert(0, "/opt/trn_rl_repo")

import numpy as np

import concourse.bass as bass
import concourse.tile as tile
from concourse import bacc, mybir
from concourse.bass_utils import run_bass_kernel_spmd

BS = 2
N_TOK = 2048
NH = 8
W = 64
S = 32  # dst_mxlen
HW = NH * W  # 512 values per token row
N_CORES = 8
DT = N_TOK // N_CORES  # dst tokens per core = 256
CHUNKS = DT // 128  # dst chunks of 128 per core = 2
CQ = HW // 128  # 4 w-lane groups (transposed gather rows per partition)
KG = 8  # k gathers per chunk (512 edges each = 4 slots)
KS = S // KG  # 4 slots per k gather
KNI = 128 * KS  # 512
VG = 4  # v gathers per chunk (1024 edges each = 8 slots)
VS = S // VG  # 8 slots per v gather
VNI = 128 * VS  # 1024


def _wrap_idx(flat):
    """int16 index list -> [128, n/16] tile layout: idx i at [i%16, i//16],
    replicated down the 8 groups of 16 partitions."""
    n = flat.shape[0]
    w16 = np.zeros((16, n // 16), dtype=np.int16)
    w16[np.arange(n) % 16, np.arange(n) // 16] = flat
    return np.tile(w16, (8, 1))


def build_kernel():
    nc = bacc.Bacc(
        "TRN2", target_bir_lowering=False, debug=False, num_devices=N_CORES,
        dynamic_dma_scratch_size=16384 * 4,
    )
    f16 = mybir.dt.float16
    f32 = mybir.dt.float32
    i16 = mybir.dt.int16
    A = mybir.AluOpType

    kt = nc.dram_tensor("kt", [BS * N_TOK, HW], f16, kind="ExternalInput").ap()
    vt = nc.dram_tensor("vt", [BS * N_TOK, HW], f16, kind="ExternalInput").ap()
    # cst packs [qT (b c n) | msk | idn] along the free dim per partition
    NIT = BS * CHUNKS
    CSTW = NIT * HW + CQ * NH + 128
    cst = nc.dram_tensor("cst", [128, CSTW], f16, kind="ExternalInput").ap()
    idx = nc.dram_tensor(
        "idx", [BS, CHUNKS, 128, S * 128 // 16], i16, kind="ExternalInput"
    ).ap()
    oc = nc.dram_tensor("oc", [BS, CHUNKS, 128, HW], f16, kind="ExternalOutput").ap()

    with tile.TileContext(nc) as tc:
        with (
            tc.tile_pool(name="kp", bufs=8) as kp,       # 0.5MB k gather tiles
            tc.tile_pool(name="vp", bufs=5) as vp,       # 1MB v gather tiles
            tc.tile_pool(name="sp", bufs=10) as sp,      # small tiles
            tc.tile_pool(name="ip", bufs=3) as ip,       # idx tiles
            tc.psum_pool(name="pp", bufs=2) as pp,       # scores
        ):
            # tiny first-granule idx lands first so gather 0 preps ASAP
            it0 = ip.tile([128, KNI // 16], i16, tag="idx0", bufs=1)
            nc.sync.dma_start(out=it0[:], in_=idx[0, 0][:, : KNI // 16])
            it_all = ip.tile([128, NIT, S * 128 // 16], i16, tag="idx", bufs=1)
            nc.sync.dma_start(
                out=it_all[:], in_=idx.rearrange("b c p n -> p (b c) n")
            )
            cst_t = sp.tile([128, CSTW], f16, tag="cst", bufs=1)
            nc.sync.dma_start(out=cst_t[:], in_=cst)
            q_all = cst_t[:, : NIT * HW].rearrange("p (i n) -> p i n", n=HW)
            msk_t = cst_t[:, NIT * HW : NIT * HW + CQ * NH]
            id_t = cst_t[:, NIT * HW + CQ * NH :]

            for b in range(BS):
                for c in range(CHUNKS):
                    bc = b * CHUNKS + c
                    it = it_all[:, bc]
                    itv = it_all[:, bc].rearrange("p (g n) -> p g n", n=KNI // 16)
                    qv = q_all[:, bc].rearrange("p (cq d) -> p cq d", d=128)[
                        :, :, None, :
                    ].to_broadcast([128, CQ, KS, 128])

                    psum = pp.tile([128, S * NH], f32, tag="L")
                    E = sp.tile([128, S * NH], f16, tag="E")

                    def k_granule(g):
                        kg = kp.tile([128, CQ, KNI], f16, tag="kg")
                        src_idx = it0[:] if bc == 0 and g == 0 else itv[:, g]
                        nc.gpsimd.dma_gather(
                            kg[:], kt, src_idx, KNI, KNI, HW,
                            transpose=True, queue_num=0,
                        )
                        kv4 = kg[:].rearrange("p cq (s d) -> p cq s d", d=128)
                        # D = k/8 - q/8 (in place on the gather tile)
                        nc.vector.tensor_tensor(
                            out=kv4, in0=kv4, in1=qv, op=A.subtract,
                        )
                        # |D| in place on the ACT engine
                        nc.scalar.activation(
                            out=kg[:], in_=kg[:],
                            func=mybir.ActivationFunctionType.Abs,
                        )
                        # PE: L[d, (s h)] += sum_w |D| via head-mask matmuls
                        for s in range(KS):
                            sl = (g * KS + s) * NH
                            for cq in range(CQ):
                                nc.tensor.matmul(
                                    psum[:, sl : sl + NH],
                                    kv4[:, cq, s, :],
                                    msk_t[:, cq * NH : (cq + 1) * NH],
                                    start=(cq == 0),
                                    stop=(cq == CQ - 1),
                                )
                        # early per-granule exp so the v side can start
                        # before the remaining score granules finish
                        gs = g * KS * NH
                        nc.scalar.activation(
                            out=E[:, gs : gs + KS * NH],
                            in_=psum[:, gs : gs + KS * NH],
                            func=mybir.ActivationFunctionType.Exp, scale=-1.0,
                        )

                    psum_o = pp.tile([128, HW], f32, tag="O")

                    def v_granule(s0, ns):
                        # gathers slots [s0, s0+ns), ns*128 indices
                        vg = vp.tile([128, ns, HW], f16, tag=f"vg{ns}")
                        nc.gpsimd.dma_gather(
                            vg[:], vt,
                            it_all[:, bc].rearrange(
                                "p (s n) -> p s n", n=128 // 16
                            )[:, s0 : s0 + ns].rearrange("p s n -> p (s n)"),
                            ns * 128, ns * 128, HW, queue_num=0,
                        )
                        ev = E[:, s0 * NH : (s0 + ns) * NH].rearrange(
                            "p (s h) -> p s h", h=NH
                        )[:, :, None, :].to_broadcast([128, ns, W, NH])
                        # weighted products in place
                        nc.vector.tensor_tensor(
                            out=vg[:].rearrange("p s (w h) -> p s w h", h=NH),
                            in0=vg[:].rearrange("p s (w h) -> p s w h", h=NH),
                            in1=ev,
                            op=A.mult,
                        )
                        # slot sum on PE: psum_o += I @ P_s
                        for s in range(ns):
                            nc.tensor.matmul(
                                psum_o[:],
                                id_t,
                                vg[:, s],
                                start=(s0 + s == 0),
                                stop=(s0 + s == S - 1),
                                skip_group_check=True,
                            )

                    # k gathers first (their compute chain is the long pole),
                    # then the v stream lands against ready score granules
                    for g in range(KG):
                        k_granule(g)

                    # normalizer (needs all score granules; consumed by the
                    # final scale only, so it hides under the v stream)
                    den = sp.tile([128, NH], f32, tag="den")
                    nc.vector.tensor_reduce(
                        out=den[:],
                        in_=E[:].rearrange("p (s h) -> p h s", h=NH),
                        axis=mybir.AxisListType.X,
                        op=A.add,
                    )
                    rden = sp.tile([128, NH], f32, tag="rden")
                    nc.vector.reciprocal(rden[:], den[:])

                    last = b == BS - 1 and c == CHUNKS - 1
                    for g in range(VG):
                        if last and g == VG - 1:
                            # split the final granule to shorten the tail
                            v_granule(g * VS, 4)
                            v_granule(g * VS + 4, 2)
                            v_granule(g * VS + 6, 1)
                            v_granule(g * VS + 7, 1)
                        else:
                            v_granule(g * VS, VS)

                    # normalize: out = psum_o * (1/den), broadcast over w
                    ot = sp.tile([128, HW], f16, tag="oc")
                    nc.vector.tensor_tensor(
                        out=ot[:].rearrange("p (w h) -> p w h", h=NH),
                        in0=psum_o[:].rearrange("p (w h) -> p w h", h=NH),
                        in1=rden[:][:, None, :].to_broadcast([128, W, NH]),
                        op=A.mult,
                    )
                    nc.sync.dma_start(out=oc[b, c], in_=ot[:])
    nc.compile()
    return nc


_NC_CACHE = None


def kernel(v, q, k, coo, dst_mxlen):
    global _NC_CACHE
    assert int(dst_mxlen) == S
    v = np.asarray(v, dtype=np.float32)
    q = np.asarray(q, dtype=np.float32)
    k = np.asarray(k, dtype=np.float32)
    coo = np.asarray(coo)

    # src table: srct[t, s] = src index of edge (dst=t, slot=s)
    srct = np.zeros((N_TOK, S), dtype=np.int64)
    srct[coo[:, 0], coo[:, 2]] = coo[:, 1]

    scale = 1.0 / np.sqrt(W)
    kt = (k * scale).astype(np.float16).reshape(BS * N_TOK, HW)
    # v table in w-major row layout: row[(w, h)] = v[h, w]
    vt = np.ascontiguousarray(v.transpose(0, 1, 3, 2)).astype(np.float16)
    vt = vt.reshape(BS * N_TOK, HW)
    qs = (q * scale).astype(np.float16).reshape(BS, N_TOK, HW)

    # PE head masks: msk[p, cq*8 + h] = 1 iff h == 2*cq + (p >= 64)
    msk = np.zeros((128, CQ * NH), dtype=np.float16)
    for cq in range(CQ):
        msk[0:64, cq * NH + 2 * cq] = 1.0
        msk[64:128, cq * NH + 2 * cq + 1] = 1.0

    if _NC_CACHE is None:
        _NC_CACHE = build_kernel()
    nc = _NC_CACHE

    in_maps = []
    for core in range(N_CORES):
        lo = core * DT
        # qT[p, (cq, d)] = q_scaled[b, lo + c*128 + d, cq*128 + p]
        qT = np.empty((BS, CHUNKS, 128, HW), dtype=np.float16)
        for b in range(BS):
            for c in range(CHUNKS):
                blk = qs[b, lo + c * 128 : lo + (c + 1) * 128]  # [128 d, 512]
                qT[b, c] = (
                    blk.reshape(128, CQ, 128).transpose(2, 1, 0).reshape(128, HW)
                )
        # edge i = s*128 + d; wrapped per 512-idx granule, concatenated
        idx = np.zeros((BS, CHUNKS, 128, S * 128 // 16), dtype=np.int16)
        for b in range(BS):
            for c in range(CHUNKS):
                flat = (
                    b * N_TOK + srct[lo + c * 128 : lo + (c + 1) * 128, :].T
                ).reshape(-1).astype(np.int16)  # [(s), (d)] flattened
                for g in range(KG):
                    idx[b, c, :, g * (KNI // 16) : (g + 1) * (KNI // 16)] = (
                        _wrap_idx(flat[g * KNI : (g + 1) * KNI])
                    )
        cst = np.concatenate(
            [
                qT.transpose(2, 0, 1, 3).reshape(128, BS * CHUNKS * HW),
                msk,
                np.eye(128, dtype=np.float16),
            ],
            axis=1,
        )
        in_maps.append({"kt": kt, "vt": vt, "cst": cst, "idx": idx})

    res = run_bass_kernel_spmd(nc, in_maps, list(range(N_CORES)))
    out = np.empty((BS, N_TOK, NH, W), dtype=np.float32)
    for core in range(N_CORES):
        lo = core * DT
        o = res.results[core]["oc"].astype(np.float32)  # [BS, CHUNKS, 128, (w h)]
        o = o.reshape(BS, CHUNKS, 128, W, NH).transpose(0, 1, 2, 4, 3)
        out[:, lo : lo + DT] = o.reshape(BS, DT, NH, W)
    return out


# revision 42
# speedup vs baseline: 1.0107x; 1.0057x over previous
"""Sparse L1-distance attention (nn_L1AttnSparse) on 8 Trainium2 NeuronCores.

Sharding: dst tokens split across the 8 cores (256 dst each, x2 batches).
Per (batch, 128-dst chunk) iteration the 4096 edges (128 dst x 32 slots)
are processed as:

  - k rows gathered fp16 TRANSPOSED ([w-lane partitions, edge free], 8
    gathers of 512 edges) so the per-(slot,head) |q-k| sum over w runs on
    the PE array: stationary = |kT - qT| slot-plane [128 w-lanes, 128 dst],
    moving = an 8-col head mask; PSUM accumulates the 4 w-lane groups and
    scores land directly as [128 dst, (slot, head)]. The subtract is a DVE
    2x fp16 TensorTensor; the abs runs on the ACT engine.
  - softmax skips the max-subtraction (L = sum|q-k|/8 <= ~25 so exp(-L)
    stays inside fp32 range): per-granule exp on ACT + sum + reciprocal.
  - v rows gathered fp16 untransposed in w-major row layout (4 gathers of
    1024 edges) so the weight broadcast keeps every operand innermost-
    packed and the weighting runs in the DVE 2x fp16 mode; the slot sum is
    32 identity-stationary matmuls accumulating into PSUM on the PE, and
    the 1/den normalization is applied once on the PSUM readout.

k and q are pre-scaled by 1/8 (= 1/sqrt(64)) on the host. All gathers stay
at <= 1024 indices (<= 512 transposed): larger SWDGE descriptor batches
crash the runtime.
"""

import sys

sys.path.insert(0, "/opt/trn_rl_repo")

import numpy as np

import concourse.bass as bass
import concourse.tile as tile
from concourse import bacc, mybir
from concourse.bass_utils import run_bass_kernel_spmd

BS = 2
N_TOK = 2048
NH = 8
W = 64
S = 32  # dst_mxlen
HW = NH * W  # 512 values per token row
N_CORES = 8
DT = N_TOK // N_CORES  # dst tokens per core = 256
CHUNKS = DT // 128  # dst chunks of 128 per core = 2
CQ = HW // 128  # 4 w-lane groups (transposed gather rows per partition)
KG = 8  # k gathers per chunk (512 edges each = 4 slots)
KS = S // KG  # 4 slots per k gather
KNI = 128 * KS  # 512
VG = 4  # v gathers per chunk (1024 edges each = 8 slots)
VS = S // VG  # 8 slots per v gather
VNI = 128 * VS  # 1024


def _wrap_idx(flat):
    """int16 index list -> [128, n/16] tile layout: idx i at [i%16, i//16],
    replicated down the 8 groups of 16 partitions."""
    n = flat.shape[0]
    w16 = np.zeros((16, n // 16), dtype=np.int16)
    w16[np.arange(n) % 16, np.arange(n) // 16] = flat
    return np.tile(w16, (8, 1))


def build_kernel():
    nc = bacc.Bacc(
        "TRN2", target_bir_lowering=False, debug=False, num_devices=N_CORES,
        dynamic_dma_scratch_size=16384 * 4,
    )
    f16 = mybir.dt.float16
    f32 = mybir.dt.float32
    i16 = mybir.dt.int16
    A = mybir.AluOpType

    kt = nc.dram_tensor("kt", [BS * N_TOK, HW], f16, kind="ExternalInput").ap()
    vt = nc.dram_tensor("vt", [BS * N_TOK, HW], f16, kind="ExternalInput").ap()
    # cst packs [qT (b c n) | msk | idn] along the free dim per partition
    NIT = BS * CHUNKS
    CSTW = NIT * HW + CQ * NH + 128
    cst = nc.dram_tensor("cst", [128, CSTW], f16, kind="ExternalInput").ap()
    idx = nc.dram_tensor(
        "idx", [BS, CHUNKS, 128, S * 128 // 16], i16, kind="ExternalInput"
    ).ap()
    oc = nc.dram_tensor("oc", [BS, CHUNKS, 128, HW], f16, kind="ExternalOutput").ap()

    with tile.TileContext(nc) as tc:
        with (
            tc.tile_pool(name="kp", bufs=8) as kp,       # 0.5MB k gather tiles
            tc.tile_pool(name="vp", bufs=5) as vp,       # 1MB v gather tiles
            tc.tile_pool(name="sp", bufs=10) as sp,      # small tiles
            tc.tile_pool(name="ip", bufs=3) as ip,       # idx tiles
            tc.psum_pool(name="pp", bufs=2) as pp,       # scores
        ):
            # tiny first-granule idx lands first so gather 0 preps ASAP
            it0 = ip.tile([128, KNI // 16], i16, tag="idx0", bufs=1)
            nc.sync.dma_start(out=it0[:], in_=idx[0, 0][:, : KNI // 16])
            it_all = ip.tile([128, NIT, S * 128 // 16], i16, tag="idx", bufs=1)
            nc.sync.dma_start(
                out=it_all[:], in_=idx.rearrange("b c p n -> p (b c) n")
            )
            cst_t = sp.tile([128, CSTW], f16, tag="cst", bufs=1)
            nc.sync.dma_start(out=cst_t[:], in_=cst)
            q_all = cst_t[:, : NIT * HW].rearrange("p (i n) -> p i n", n=HW)
            msk_t = cst_t[:, NIT * HW : NIT * HW + CQ * NH]
            id_t = cst_t[:, NIT * HW + CQ * NH :]

            for b in range(BS):
                for c in range(CHUNKS):
                    bc = b * CHUNKS + c
                    it = it_all[:, bc]
                    itv = it_all[:, bc].rearrange("p (g n) -> p g n", n=KNI // 16)
                    qv = q_all[:, bc].rearrange("p (cq d) -> p cq d", d=128)[
                        :, :, None, :
                    ].to_broadcast([128, CQ, KS, 128])

                    psum = pp.tile([128, S * NH], f32, tag="L")
                    E = sp.tile([128, S * NH], f16, tag="E")

                    def k_granule(g):
                        kg = kp.tile([128, CQ, KNI], f16, tag="kg")
                        src_idx = it0[:] if bc == 0 and g == 0 else itv[:, g]
                        nc.gpsimd.dma_gather(
                            kg[:], kt, src_idx, KNI, KNI, HW,
                            transpose=True, queue_num=0,
                        )
                        kv4 = kg[:].rearrange("p cq (s d) -> p cq s d", d=128)
                        # D = k/8 - q/8 (in place on the gather tile)
                        nc.vector.tensor_tensor(
                            out=kv4, in0=kv4, in1=qv, op=A.subtract,
                        )
                        # |D| in place on the ACT engine
                        nc.scalar.activation(
                            out=kg[:], in_=kg[:],
                            func=mybir.ActivationFunctionType.Abs,
                        )
                        # PE: L[d, (s h)] += sum_w |D| via head-mask matmuls
                        for s in range(KS):
                            sl = (g * KS + s) * NH
                            for cq in range(CQ):
                                nc.tensor.matmul(
                                    psum[:, sl : sl + NH],
                                    kv4[:, cq, s, :],
                                    msk_t[:, cq * NH : (cq + 1) * NH],
                                    start=(cq == 0),
                                    stop=(cq == CQ - 1),
                                )
                        # early per-granule exp so the v side can start
                        # before the remaining score granules finish
                        gs = g * KS * NH
                        nc.scalar.activation(
                            out=E[:, gs : gs + KS * NH],
                            in_=psum[:, gs : gs + KS * NH],
                            func=mybir.ActivationFunctionType.Exp, scale=-1.0,
                        )

                    psum_o = pp.tile([128, HW], f32, tag="O")

                    def v_granule(s0, ns):
                        # gathers slots [s0, s0+ns), ns*128 indices
                        vg = vp.tile([128, ns, HW], f16, tag=f"vg{ns}")
                        nc.gpsimd.dma_gather(
                            vg[:], vt,
                            it_all[:, bc].rearrange(
                                "p (s n) -> p s n", n=128 // 16
                            )[:, s0 : s0 + ns].rearrange("p s n -> p (s n)"),
                            ns * 128, ns * 128, HW, queue_num=0,
                        )
                        ev = E[:, s0 * NH : (s0 + ns) * NH].rearrange(
                            "p (s h) -> p s h", h=NH
                        )[:, :, None, :].to_broadcast([128, ns, W, NH])
                        # weighted products in place
                        nc.vector.tensor_tensor(
                            out=vg[:].rearrange("p s (w h) -> p s w h", h=NH),
                            in0=vg[:].rearrange("p s (w h) -> p s w h", h=NH),
                            in1=ev,
                            op=A.mult,
                        )
                        # slot sum on PE: psum_o += I @ P_s
                        for s in range(ns):
                            nc.tensor.matmul(
                                psum_o[:],
                                id_t,
                                vg[:, s],
                                start=(s0 + s == 0),
                                stop=(s0 + s == S - 1),
                                skip_group_check=True,
                            )

                    # k gathers first (their compute chain is the long pole),
                    # then the v stream lands against ready score granules
                    for g in range(KG):
                        k_granule(g)

                    # normalizer (needs all score granules; consumed by the
                    # final scale only, so it hides under the v stream)
                    den = sp.tile([128, NH], f32, tag="den")
                    nc.vector.tensor_reduce(
                        out=den[:],
                        in_=E[:].rearrange("p (s h) -> p h s", h=NH),
                        axis=mybir.AxisListType.X,
                        op=A.add,
                    )
                    rden = sp.tile([128, NH], f32, tag="rden")
                    nc.vector.reciprocal(rden[:], den[:])

                    last = b == BS - 1 and c == CHUNKS - 1
                    for g in range(VG):
                        if last and g == VG - 2:
                            v_granule(g * VS, 4)
                            v_granule(g * VS + 4, 4)
                        elif last and g == VG - 1:
                            # split the final granule to shorten the tail
                            v_granule(g * VS, 4)
                            v_granule(g * VS + 4, 2)
                            v_granule(g * VS + 6, 1)
                            v_granule(g * VS + 7, 1)
                        else:
                            v_granule(g * VS, VS)

                    # normalize: out = psum_o * (1/den), broadcast over w
                    ot = sp.tile([128, HW], f16, tag="oc")
                    nc.vector.tensor_tensor(
                        out=ot[:].rearrange("p (w h) -> p w h", h=NH),
                        in0=psum_o[:].rearrange("p (w h) -> p w h", h=NH),
                        in1=rden[:][:, None, :].to_broadcast([128, W, NH]),
                        op=A.mult,
                    )
                    nc.sync.dma_start(out=oc[b, c], in_=ot[:])
    nc.compile()
    return nc


_NC_CACHE = None


def kernel(v, q, k, coo, dst_mxlen):
    global _NC_CACHE
    assert int(dst_mxlen) == S
    v = np.asarray(v, dtype=np.float32)
    q = np.asarray(q, dtype=np.float32)
    k = np.asarray(k, dtype=np.float32)
    coo = np.asarray(coo)

    # src table: srct[t, s] = src index of edge (dst=t, slot=s)
    srct = np.zeros((N_TOK, S), dtype=np.int64)
    srct[coo[:, 0], coo[:, 2]] = coo[:, 1]

    scale = 1.0 / np.sqrt(W)
    kt = (k * scale).astype(np.float16).reshape(BS * N_TOK, HW)
    # v table in w-major row layout: row[(w, h)] = v[h, w]
    vt = np.ascontiguousarray(v.transpose(0, 1, 3, 2)).astype(np.float16)
    vt = vt.reshape(BS * N_TOK, HW)
    qs = (q * scale).astype(np.float16).reshape(BS, N_TOK, HW)

    # PE head masks: msk[p, cq*8 + h] = 1 iff h == 2*cq + (p >= 64)
    msk = np.zeros((128, CQ * NH), dtype=np.float16)
    for cq in range(CQ):
        msk[0:64, cq * NH + 2 * cq] = 1.0
        msk[64:128, cq * NH + 2 * cq + 1] = 1.0

    if _NC_CACHE is None:
        _NC_CACHE = build_kernel()
    nc = _NC_CACHE

    in_maps = []
    for core in range(N_CORES):
        lo = core * DT
        # qT[p, (cq, d)] = q_scaled[b, lo + c*128 + d, cq*128 + p]
        qT = np.empty((BS, CHUNKS, 128, HW), dtype=np.float16)
        for b in range(BS):
            for c in range(CHUNKS):
                blk = qs[b, lo + c * 128 : lo + (c + 1) * 128]  # [128 d, 512]
                qT[b, c] = (
                    blk.reshape(128, CQ, 128).transpose(2, 1, 0).reshape(128, HW)
                )
        # edge i = s*128 + d; wrapped per 512-idx granule, concatenated
        idx = np.zeros((BS, CHUNKS, 128, S * 128 // 16), dtype=np.int16)
        for b in range(BS):
            for c in range(CHUNKS):
                flat = (
                    b * N_TOK + srct[lo + c * 128 : lo + (c + 1) * 128, :].T
                ).reshape(-1).astype(np.int16)  # [(s), (d)] flattened
                for g in range(KG):
                    idx[b, c, :, g * (KNI // 16) : (g + 1) * (KNI // 16)] = (
                        _wrap_idx(flat[g * KNI : (g + 1) * KNI])
                    )
        cst = np.concatenate(
            [
                qT.transpose(2, 0, 1, 3).reshape(128, BS * CHUNKS * HW),
                msk,
                np.eye(128, dtype=np.float16),
            ],
            axis=1,
        )
        in_maps.append({"kt": kt, "vt": vt, "cst": cst, "idx": idx})

    res = run_bass_kernel_spmd(nc, in_maps, list(range(N_CORES)))
    out = np.empty((BS, N_TOK, NH, W), dtype=np.float32)
    for core in range(N_CORES):
        lo = core * DT
        o = res.results[core]["oc"].astype(np.float32)  # [BS, CHUNKS, 128, (w h)]
        o = o.reshape(BS, CHUNKS, 128, W, NH).transpose(0, 1, 2, 4, 3)
        out[:, lo : lo + DT] = o.reshape(BS, DT, NH, W)
    return out
